# revision 1
# baseline (speedup 1.0000x reference)
"""Single-head causal self-attention (B=8, T=2048, D=512, H=64), data-parallel
over batch across 8 NeuronCores. Self-contained: builds a Bass/Tile kernel and
runs it via run_bass_kernel_spmd.

Per-core layout (batch element b = core id):
  - x [2048, 512] is PE-transposed to xT (d on partitions, f32r)
  - kT/qT [64, 2048] and v [2048, 64] projections in f32r; Wq, bq pre-scaled
    by H^-0.5 on the host; v is augmented with a ones column so the PV matmul
    also accumulates the softmax denominator
  - attention runs in S^T layout per 512-wide i-block: S^T = kT_chunk^T @ qT,
    exp on ACT (PSUM->SBUF, two j-tiles per instruction), multiplicative
    causal masks on the 4 diagonal j-tiles, PV matmul accumulates [65, 512]
  - epilogue: PE transpose of [65, 128] chunks, reciprocal * row, + bv
    (softmax rows sum to 1, so the v bias folds into the output)
"""

import sys

for _p in ("/root/.axon_site/_ro/trn_rl_repo", "/opt/trn_rl_repo"):
    if _p not in sys.path:
        sys.path.append(_p)

import numpy as np
import concourse.bass as bass
import concourse.bacc as bacc
import concourse.tile as tile
from concourse import mybir
from concourse.bass_utils import run_bass_kernel_spmd
from concourse.masks import make_identity

F32 = mybir.dt.float32
F32R = mybir.dt.float32r

B, T, D, H = 8, 2048, 512, 64
NT = T // 128   # 16 t-tiles
ND = D // 128   # 4 d-chunks
NIB = T // 512  # 4 i-blocks
EXP = mybir.ActivationFunctionType.Exp


def build_body(nc, tc, ctx, dram, repeat=1):
    x_d, w_d, bkq_d, bv_d, out_d = dram

    persist = ctx.enter_context(tc.tile_pool(name="persist", bufs=1))
    epool = ctx.enter_context(tc.tile_pool(name="epool", bufs=6))
    otspool = ctx.enter_context(tc.tile_pool(name="otspool", bufs=3))
    opool = ctx.enter_context(tc.tile_pool(name="opool", bufs=3))
    rpool = ctx.enter_context(tc.tile_pool(name="rpool", bufs=3))
    pspool = ctx.enter_context(tc.tile_pool(name="ps", bufs=2, space="PSUM"))
    ps2pool = ctx.enter_context(tc.tile_pool(name="ps2", bufs=2, space="PSUM"))
    otppool = ctx.enter_context(tc.tile_pool(name="otp", bufs=2, space="PSUM"))

    # --- constants ---
    ident = persist.tile([128, 128], F32)
    make_identity(nc, ident[:])

    bkq_sb = persist.tile([64, 2], F32)
    bv_row = persist.tile([1, 64], F32)
    bvB = persist.tile([128, 64], F32)
    nc.gpsimd.dma_start(bkq_sb[:], bkq_d[:])
    nc.gpsimd.dma_start(bv_row[:], bv_d[:])
    nc.gpsimd.partition_broadcast(bvB[:], bv_row[:])

    # weights -> f32r (packed [ND, 128, 3*64]: k | q | v along last axis)
    wstage = persist.tile([128, ND, 3 * H], F32)
    nc.gpsimd.dma_start(wstage[:], w_d.rearrange("a p h -> p a h"))
    w_r = persist.tile([128, ND, 3 * H], F32R)
    nc.vector.tensor_copy(w_r[:], wstage[:])

    ones_col = persist.tile([128, 1], F32)
    nc.vector.memset(ones_col[:], 1.0)

    # persistent activations
    x_all = persist.tile([128, NT, D], F32)
    xT = persist.tile([128, ND, T], F32R)     # x transposed, d on partitions
    kT = persist.tile([64, T], F32R)
    qT = persist.tile([64, T], F32R)
    vTs = persist.tile([64, T], F32)
    v_aug = persist.tile([128, NT, 65], F32R)  # v rows + ones column
    o_all = persist.tile([128, NT, 64], F32)

    for rep in range(repeat):
        for jt in range(NT):
            nc.vector.tensor_copy(v_aug[:, jt, 64:65], ones_col[:])

        # x in: staged DMAs, small first so transposes start early
        t0 = 0
        for gi, ntile in enumerate((2, 2, 4, 4, 4)):
            eng = nc.sync if gi % 2 == 0 else nc.scalar
            eng.dma_start(
                x_all[:, t0:t0 + ntile, :],
                x_d[128 * t0:128 * (t0 + ntile), :].rearrange(
                    "(a p) d -> p a d", p=128),
            )
            t0 += ntile

        # per 512-wide t-chunk: transpose x, project k/q/v, build v_aug
        for tch in range(4):
            tsl = slice(tch * 512, (tch + 1) * 512)
            for dc in range(ND):
                tp = pspool.tile([128, 4, 128], F32, tag="ps")
                for q in range(4):
                    ti = 4 * tch + q
                    nc.tensor.transpose(
                        tp[:, q, :], x_all[:, ti, dc * 128:(dc + 1) * 128],
                        ident[:])
                nc.vector.tensor_copy(xT[:, dc, tsl], tp[:])

            k_ps = pspool.tile([64, 512], F32, tag="ps")
            for dc in range(ND):
                nc.tensor.matmul(k_ps[:], w_r[:, dc, 0:64], xT[:, dc, tsl],
                                 start=(dc == 0), stop=(dc == ND - 1))
            nc.vector.tensor_scalar_add(kT[:, tsl], k_ps[:], bkq_sb[:, 0:1])

            q_ps = pspool.tile([64, 512], F32, tag="ps")
            for dc in range(ND):
                nc.tensor.matmul(q_ps[:], w_r[:, dc, 64:128], xT[:, dc, tsl],
                                 start=(dc == 0), stop=(dc == ND - 1))
            nc.vector.tensor_scalar_add(qT[:, tsl], q_ps[:], bkq_sb[:, 1:2])

            v_ps = pspool.tile([64, 512], F32, tag="ps")
            for dc in range(ND):
                nc.tensor.matmul(v_ps[:], w_r[:, dc, 128:192], xT[:, dc, tsl],
                                 start=(dc == 0), stop=(dc == ND - 1))
            nc.vector.tensor_copy(vTs[:, tsl], v_ps[:])

            va_ps = pspool.tile([128, 4, 64], F32, tag="ps")
            for q in range(4):
                jt = 4 * tch + q
                nc.tensor.transpose(va_ps[:, q, :], vTs[:, jt * 128:(jt + 1) * 128],
                                    ident[0:64, 0:64])
            nc.vector.tensor_copy(v_aug[:, 4 * tch:4 * tch + 4, 0:64], va_ps[:])

        # --- attention per 512-wide i-block, S^T layout, j-tiles in pairs ---
        for bi in range(NIB):
            isl = slice(bi * 512, (bi + 1) * 512)
            njt = 4 * (bi + 1)
            ot_ps = otppool.tile([65, 512], F32, tag="ot")
            for jp in range(njt // 2):
                st2 = ps2pool.tile([128, 2, 512], F32, tag="ps2")
                for h in range(2):
                    jt = 2 * jp + h
                    nc.tensor.matmul(st2[:, h, :], kT[:, jt * 128:(jt + 1) * 128],
                                     qT[:, isl], start=True, stop=True)
                e2 = epool.tile([128, 2, 512], F32R, tag="e")
                nc.scalar.activation(e2[:], st2[:], EXP)
                for h in range(2):
                    jt = 2 * jp + h
                    if jt >= 4 * bi:
                        nc.gpsimd.affine_select(
                            out=e2[:, h, :], in_=e2[:, h, :],
                            compare_op=mybir.AluOpType.is_ge, fill=0.0,
                            base=-128 * (jt - 4 * bi),
                            pattern=[[1, 512]], channel_multiplier=-1)
                    nc.tensor.matmul(ot_ps[:], v_aug[:, jt, :], e2[:, h, :],
                                     start=(jt == 0), stop=(jt == njt - 1))

            ots = otspool.tile([65, 512], F32, tag="ots")
            nc.vector.tensor_copy(ots[:], ot_ps[:])
            for c in range(4):
                o_ps = pspool.tile([128, 65], F32, tag="ps")
                nc.tensor.transpose(o_ps[:], ots[:, c * 128:(c + 1) * 128],
                                    ident[0:65, 0:65])
                rec = rpool.tile([128, 1], F32, tag="r")
                nc.vector.reciprocal(rec[:], o_ps[:, 64:65])
                it = bi * 4 + c
                nc.vector.tensor_scalar_mul(o_all[:, it, :], o_ps[:, 0:64], rec[:])
                nc.vector.tensor_add(o_all[:, it, :], o_all[:, it, :], bvB[:])

        # output DMA per i-block so the store drains while later blocks run
        for bi in range(NIB):
            nc.sync.dma_start(
                out_d[512 * bi:512 * (bi + 1), :].rearrange(
                    "(a p) h -> p a h", p=128),
                o_all[:, 4 * bi:4 * bi + 4, :])


def build_nc(repeat=1):
    nc = bacc.Bacc("TRN2", target_bir_lowering=False, debug=False, num_devices=8)
    x_d = nc.dram_tensor("x", [T, D], F32, kind="ExternalInput")
    w_d = nc.dram_tensor("w", [ND, 128, 3 * H], F32, kind="ExternalInput")
    bkq_d = nc.dram_tensor("bkq", [H, 2], F32, kind="ExternalInput")
    bv_d = nc.dram_tensor("bv", [1, H], F32, kind="ExternalInput")
    out_d = nc.dram_tensor("out", [T, H], F32, kind="ExternalOutput")
    dram = (x_d, w_d, bkq_d, bv_d, out_d)

    from contextlib import ExitStack
    with tile.TileContext(nc) as tc:
        with ExitStack() as ctx:
            build_body(nc, tc, ctx, dram, repeat=repeat)
    nc.compile()
    return nc


_NC_CACHE = {}


def _get_nc(repeat=1):
    if repeat not in _NC_CACHE:
        _NC_CACHE[repeat] = build_nc(repeat)
    return _NC_CACHE[repeat]


def make_in_maps(x, Wk, bk, Wq, bq, Wv, bv):
    scale = float(H) ** -0.5
    w = np.concatenate(
        [Wk.reshape(ND, 128, H), (Wq * scale).reshape(ND, 128, H),
         Wv.reshape(ND, 128, H)], axis=2)
    w = np.ascontiguousarray(w)
    bkq = np.ascontiguousarray(np.stack([bk, bq * scale], axis=1))
    bvr = np.ascontiguousarray(bv.reshape(1, H))
    return [
        {"x": np.ascontiguousarray(x[b]), "w": w, "bkq": bkq, "bv": bvr}
        for b in range(B)
    ]


def kernel(x, Wk, bk, Wq, bq, Wv, bv, _repeat=1):
    x = np.asarray(x, dtype=np.float32)
    Wk = np.asarray(Wk, dtype=np.float32)
    bk = np.asarray(bk, dtype=np.float32)
    Wq = np.asarray(Wq, dtype=np.float32)
    bq = np.asarray(bq, dtype=np.float32)
    Wv = np.asarray(Wv, dtype=np.float32)
    bv = np.asarray(bv, dtype=np.float32)

    nc = _get_nc(_repeat)
    in_maps = make_in_maps(x, Wk, bk, Wq, bq, Wv, bv)
    res = run_bass_kernel_spmd(nc, in_maps, core_ids=list(range(B)))
    out = np.stack([res.results[b]["out"] for b in range(B)], axis=0)
    return out.astype(np.float32)



# revision 10
# speedup vs baseline: 1.5510x; 1.5510x over previous
"""Single-head causal self-attention (B=8, T=2048, D=512, H=64), data-parallel
over batch across 8 NeuronCores. Self-contained: builds a Bass/Tile kernel and
runs it via run_bass_kernel_spmd.

v2 design (per core, batch element b = core id), all-bf16 compute:
  - host prep: x is transposed + cast to bf16 (xT [4,128,2048]); Wk|Wq*s|Wv
    packed [4,128,192] bf16; bv pre-broadcast [128,64]; causal mask tiles
    [128,2,256] precomputed
  - projections per t-tile: stationary xT chunk [128d,128t] x moving W
    [128d,192] -> psum [128t, k|q|v]; v (+bv) goes straight to v_aug bf16;
    k,q are PE-transposed (bf16 identity) into [64,t] strips, then copied
    to kT/qT sbuf with per-partition bias add
  - attention per 256-wide i-block: S^T = kT_j^T @ qT_block in groups of 4
    j-tiles into one 2-bank psum region; one exp per group (ACT) -> e2 bf16;
    diagonal masks via precomputed bf16 mask multiply (DVE); PV uses e2
    chunks as stationary and v_aug [128,65] as moving -> out [t,65] psum
    accumulated over j; ones column gives the softmax denominator
  - epilogue per i-block: reciprocal of col 64, scale, DMA out [t,h] fp32
"""

import sys

for _p in ("/root/.axon_site/_ro/trn_rl_repo", "/opt/trn_rl_repo"):
    if _p not in sys.path:
        sys.path.append(_p)

import numpy as np
import concourse.bass as bass
import concourse.bacc as bacc
import concourse.tile as tile
from concourse import mybir
from concourse.bass_utils import run_bass_kernel_spmd
from concourse.masks import make_identity

F32 = mybir.dt.float32
BF16 = mybir.dt.bfloat16

B, T, D, H = 8, 2048, 512, 64
NT = T // 128   # 16 t-tiles
ND = D // 128   # 4 d-chunks
NIB = T // 256  # 8 i-blocks
EXP = mybir.ActivationFunctionType.Exp
MULT = mybir.AluOpType.mult
ADD = mybir.AluOpType.add


def build_body(nc, tc, ctx, dram, repeat=1):
    xT_d, w_d, bkq_d, bvB_d, mask_d, out_d = dram

    persist = ctx.enter_context(tc.tile_pool(name="persist", bufs=1))
    stg = ctx.enter_context(tc.tile_pool(name="stg", bufs=3))
    e2pool = ctx.enter_context(tc.tile_pool(name="e2", bufs=3))
    recpool = ctx.enter_context(tc.tile_pool(name="rec", bufs=2))
    psKQV = ctx.enter_context(tc.tile_pool(name="psKQV", bufs=1, space="PSUM"))
    psKQT = ctx.enter_context(tc.tile_pool(name="psKQT", bufs=1, space="PSUM"))
    psS = ctx.enter_context(tc.tile_pool(name="psS", bufs=2, space="PSUM"))
    psO = ctx.enter_context(tc.tile_pool(name="psO", bufs=2, space="PSUM"))

    identB = persist.tile([128, 128], BF16)
    make_identity(nc, identB[:])

    xT = persist.tile([128, ND, T], BF16)
    w_sb = persist.tile([128, ND, 3 * H], BF16)
    bkq = persist.tile([64, 2], F32)
    bvB = persist.tile([128, 64], BF16)
    masks = persist.tile([128, 2, 256], BF16)
    kqT = persist.tile([64, 2, T], BF16)      # [h, {k,q}, t]
    v_aug = persist.tile([128, NT, 65], BF16)  # v rows (+bv) | ones column
    o_all = persist.tile([128, NT, 64], F32)

    # small/constant DMAs on the gpsimd queue
    nc.gpsimd.dma_start(w_sb[:], w_d.rearrange("a p h -> p a h"))
    nc.gpsimd.dma_start(bkq[:], bkq_d[:])
    nc.gpsimd.dma_start(bvB[:], bvB_d[:])
    nc.gpsimd.dma_start(masks[:], mask_d[:])

    # x input: staged t-spans, small first so projections start early
    t0 = 0
    for gi, ntile in enumerate((1, 1, 2, 4, 4, 4)):
        eng = nc.sync if gi % 2 == 0 else nc.scalar
        eng.dma_start(
            xT[:, :, 128 * t0:128 * (t0 + ntile)],
            xT_d[:, :, 128 * t0:128 * (t0 + ntile)].rearrange("a p t -> p a t"),
        )
        t0 += ntile

    strip = [None]

    def proj(tt):
        ps = psKQV.tile([128, 3 * H], F32, tag="kqv")
        for dc in range(ND):
            nc.tensor.matmul(ps[:], xT[:, dc, tt * 128:(tt + 1) * 128],
                             w_sb[:, dc, :], start=(dc == 0), stop=(dc == ND - 1))
        st = stg.tile([128, 128], BF16, tag="stage")
        nc.vector.tensor_copy(st[:], ps[:, 0:128])
        nc.vector.tensor_tensor(v_aug[:, tt, 0:64], ps[:, 128:192], bvB[:], ADD)
        if tt % 2 == 0:
            strip[0] = psKQT.tile([64, 2, 256], BF16, tag="kqT", name="kqT_strip")
        half = (tt % 2) * 128
        nc.tensor.transpose(strip[0][:, 0, half:half + 128], st[:, 0:64], identB[:])
        nc.tensor.transpose(strip[0][:, 1, half:half + 128], st[:, 64:128], identB[:])
        if tt % 2 == 1:
            tsl = slice((tt - 1) * 128, (tt + 1) * 128)
            nc.vector.tensor_scalar_add(kqT[:, 0, tsl], strip[0][:, 0, :], bkq[:, 0:1])
            nc.vector.tensor_scalar_add(kqT[:, 1, tsl], strip[0][:, 1, :], bkq[:, 1:2])

    def attn(ib):
        njt = 2 * ib + 2
        isl = slice(ib * 256, (ib + 1) * 256)
        po = [psO.tile([128, 65], F32, tag="O", name=f"po{c}") for c in range(2)]
        for g0 in range(0, njt, 4):
            nv = min(4, njt - g0)
            ps = psS.tile([128, 4, 256], F32, tag="S")
            for s in range(nv):
                jt = g0 + s
                nc.tensor.matmul(ps[:, s, :], kqT[:, 0, jt * 128:(jt + 1) * 128],
                                 kqT[:, 1, isl], start=True, stop=True)
            e2 = e2pool.tile([128, 4, 256], BF16, tag="e2")
            nc.scalar.activation(e2[:, 0:nv, :], ps[:, 0:nv, :], EXP)
            if g0 + nv == njt:
                nc.vector.tensor_tensor(e2[:, nv - 2:nv, :], e2[:, nv - 2:nv, :],
                                        masks[:], MULT)
            for s in range(nv):
                jt = g0 + s
                for c in range(2):
                    nc.tensor.matmul(po[c][:], e2[:, s, c * 128:(c + 1) * 128],
                                     v_aug[:, jt, :],
                                     start=(jt == 0), stop=(jt == njt - 1))
        for c in range(2):
            rec = recpool.tile([128, 1], F32, tag="rec", name=f"rec{c}")
            nc.vector.reciprocal(rec[:], po[c][:, 64:65])
            nc.vector.tensor_scalar_mul(o_all[:, 2 * ib + c, :], po[c][:, 0:64],
                                        rec[:])
        nc.sync.dma_start(
            out_d[256 * ib:256 * (ib + 1), :].rearrange("(a p) h -> p a h", p=128),
            o_all[:, 2 * ib:2 * ib + 2, :])

    for rep in range(repeat):
        nc.vector.memset(v_aug[:, :, 64:65], 1.0)
        for tt in range(NT):
            proj(tt)
            if tt % 2 == 1:
                attn(tt // 2)


def build_nc(repeat=1):
    nc = bacc.Bacc("TRN2", target_bir_lowering=False, debug=False, num_devices=8)
    xT_d = nc.dram_tensor("xT", [ND, 128, T], BF16, kind="ExternalInput")
    w_d = nc.dram_tensor("w", [ND, 128, 3 * H], BF16, kind="ExternalInput")
    bkq_d = nc.dram_tensor("bkq", [H, 2], F32, kind="ExternalInput")
    bvB_d = nc.dram_tensor("bvB", [128, H], BF16, kind="ExternalInput")
    mask_d = nc.dram_tensor("mask", [128, 2, 256], BF16, kind="ExternalInput")
    out_d = nc.dram_tensor("out", [T, H], F32, kind="ExternalOutput")
    dram = (xT_d, w_d, bkq_d, bvB_d, mask_d, out_d)

    from contextlib import ExitStack
    with tile.TileContext(nc) as tc:
        with ExitStack() as ctx:
            build_body(nc, tc, ctx, dram, repeat=repeat)
    nc.compile()
    return nc


_NC_CACHE = {}


def _get_nc(repeat=1):
    if repeat not in _NC_CACHE:
        _NC_CACHE[repeat] = build_nc(repeat)
    return _NC_CACHE[repeat]


def make_in_maps(x, Wk, bk, Wq, bq, Wv, bv):
    import ml_dtypes
    bf16 = ml_dtypes.bfloat16
    scale = float(H) ** -0.5
    w = np.concatenate([Wk, Wq * scale, Wv], axis=1)          # [512, 192]
    w = np.ascontiguousarray(w.reshape(ND, 128, 3 * H)).astype(bf16)
    bkq = np.ascontiguousarray(np.stack([bk, bq * scale], axis=1)).astype(np.float32)
    bvB = np.ascontiguousarray(np.broadcast_to(bv, (128, H))).astype(bf16)
    r = np.arange(128)[:, None]
    c = np.arange(256)[None, :]
    masks = np.stack([(c >= r), (c >= r + 128)], axis=1).astype(bf16)  # [128,2,256]
    ins = []
    for b in range(B):
        xTb = np.ascontiguousarray(x[b].T).astype(bf16).reshape(ND, 128, T)
        ins.append({"xT": xTb, "w": w, "bkq": bkq, "bvB": bvB, "mask": masks})
    return ins


def kernel(x, Wk, bk, Wq, bq, Wv, bv, _repeat=1):
    x = np.asarray(x, dtype=np.float32)
    Wk = np.asarray(Wk, dtype=np.float32)
    bk = np.asarray(bk, dtype=np.float32)
    Wq = np.asarray(Wq, dtype=np.float32)
    bq = np.asarray(bq, dtype=np.float32)
    Wv = np.asarray(Wv, dtype=np.float32)
    bv = np.asarray(bv, dtype=np.float32)

    nc = _get_nc(_repeat)
    in_maps = make_in_maps(x, Wk, bk, Wq, bq, Wv, bv)
    res = run_bass_kernel_spmd(nc, in_maps, core_ids=list(range(B)))
    out = np.stack([res.results[b]["out"] for b in range(B)], axis=0)
    return out.astype(np.float32)


# revision 16
# speedup vs baseline: 1.6875x; 1.0880x over previous
"""Single-head causal self-attention (B=8, T=2048, D=512, H=64), data-parallel
over batch across 8 NeuronCores. Self-contained: builds a Bass/Tile kernel and
runs it via run_bass_kernel_spmd.

v3 design (per core, batch element b = core id), all-bf16 compute:
  - host prep: x transposed + cast to bf16 (xT [4,128,2048]); Wk|Wq*s|Wv
    packed [4,128,192] bf16; bv pre-broadcast [128,64]; triangular mask tile
    [128,2,128] precomputed
  - projections per t-tile: stationary xT chunk [128d,128t] x moving W
    [128d,192] -> psum [128t, k|q|v]; v+bv -> v_aug bf16 (gpsimd); k,q are
    PE-transposed (bf16) into [64, 2tile] psum strips, then DVE-copied to
    kT/qT sbuf with per-partition bias add
  - attention per 256-wide i-block ib (j-tiles 0..2ib+1): S^T groups packed
    into flat [128,<=1024] psum: group0 = [diag-even full 256 | diag-odd
    right-half 128 | up to 2 normal tiles], later groups 4 normal tiles;
    one exp per group (ACT) -> e2 bf16; triangular mask multiply on the two
    diagonal 128-col pieces (DVE, one strided op); PV uses e2 128-col chunks
    as stationary and v_aug [128,65] (v|1) as moving -> [t,65] psum
    accumulated over j; fully-masked diag-odd/left-half PV matmul skipped
  - epilogue per i-block: reciprocal of ones-column, scale, DMA out fp32
  - emission is software-pipelined: P/S units interleaved, V (PV) units
    lagged 2 S-units behind their exp so PE never waits on ACT
"""

import sys
from collections import deque

for _p in ("/root/.axon_site/_ro/trn_rl_repo", "/opt/trn_rl_repo"):
    if _p not in sys.path:
        sys.path.append(_p)

import numpy as np
import concourse.bass as bass
import concourse.bacc as bacc
import concourse.tile as tile
from concourse import mybir
from concourse.bass_utils import run_bass_kernel_spmd
from concourse.masks import make_identity

F32 = mybir.dt.float32
BF16 = mybir.dt.bfloat16

B, T, D, H = 8, 2048, 512, 64
NT = T // 128   # 16 t-tiles
ND = D // 128   # 4 d-chunks
NIB = T // 256  # 8 i-blocks
EXP = mybir.ActivationFunctionType.Exp
MULT = mybir.AluOpType.mult
ADD = mybir.AluOpType.add


def attn_groups(ib):
    """Group descriptors for i-block ib. Each group: list of slots
    (jt, col0, width, cs) packed into a flat <=1024-col psum region; no
    slot may cross a 512-col psum bank boundary. Normal tiles fill groups
    of 4; the final group holds <=2 normals plus the diagonal pieces:
    diag-even full 256 and diag-odd right-half 128 (its left half is
    fully masked -> PV c=0 skipped)."""
    nrm = list(range(2 * ib))  # normal (fully causal) j-tiles
    groups = []
    while len(nrm) > 2:
        groups.append([(nrm.pop(0), i * 256, 256, (0, 1)) for i in range(4)])
    last, col = [], 0
    while nrm:
        last.append((nrm.pop(0), col, 256, (0, 1)))
        col += 256
    last.append((2 * ib, col, 256, (0, 1)))       # diag-even
    last.append((2 * ib + 1, col + 256, 128, (1,)))  # diag-odd right half
    groups.append(last)
    return groups


def build_body(nc, tc, ctx, dram, repeat=1):
    xT_d, w_d, bkq_d, bvB_d, mask_d, out_d = dram

    persist = ctx.enter_context(tc.tile_pool(name="persist", bufs=1))
    stg = ctx.enter_context(tc.tile_pool(name="stg", bufs=3))
    e2pool = ctx.enter_context(tc.tile_pool(name="e2", bufs=4))
    recpool = ctx.enter_context(tc.tile_pool(name="rec", bufs=2))
    psKQV = ctx.enter_context(tc.tile_pool(name="psKQV", bufs=1, space="PSUM"))
    psKQT = ctx.enter_context(tc.tile_pool(name="psKQT", bufs=1, space="PSUM"))
    psS = ctx.enter_context(tc.tile_pool(name="psS", bufs=2, space="PSUM"))
    psO = ctx.enter_context(tc.tile_pool(name="psO", bufs=2, space="PSUM"))

    identB = persist.tile([128, 128], BF16)
    make_identity(nc, identB[:])

    xT = persist.tile([128, ND, T], BF16)
    w_sb = persist.tile([128, ND, 3 * H], BF16)
    bkq = persist.tile([64, 2], F32)
    bvB = persist.tile([128, 64], BF16)
    masks = persist.tile([128, 2, 128], BF16)
    kqT = persist.tile([64, 2, T], BF16)       # [h, {k,q}, t]
    v_aug = persist.tile([128, NT, 65], BF16)  # v rows (+bv) | ones column
    o_all = persist.tile([128, NT, 64], F32)

    # constants: scalar-engine HWDGE queue (fast fixed overhead)
    nc.scalar.dma_start(w_sb[:], w_d.rearrange("a p h -> p a h"))
    nc.scalar.dma_start(bkq[:], bkq_d[:])
    nc.scalar.dma_start(bvB[:], bvB_d[:])
    nc.scalar.dma_start(masks[:], mask_d[:])

    # x input: staged t-spans, small first so projections start early
    t0 = 0
    for gi, ntile in enumerate((1, 1, 2, 4, 4, 4)):
        eng = nc.sync if gi % 2 == 0 else nc.scalar
        eng.dma_start(
            xT[:, :, 128 * t0:128 * (t0 + ntile)],
            xT_d[:, :, 128 * t0:128 * (t0 + ntile)].rearrange("a p t -> p a t"),
        )
        t0 += ntile

    state = {"strip": None, "stage": [None, None]}

    def unit_P(tt):
        ps = psKQV.tile([128, 3 * H], F32, tag="kqv")
        for dc in range(ND):
            nc.tensor.matmul(ps[:], xT[:, dc, tt * 128:(tt + 1) * 128],
                             w_sb[:, dc, :], start=(dc == 0), stop=(dc == ND - 1))
        st = stg.tile([128, 128], BF16, tag="stage", name=f"st{tt}")
        nc.vector.tensor_copy(st[:], ps[:, 0:128])
        nc.vector.tensor_tensor(v_aug[:, tt, 0:64], ps[:, 128:192], bvB[:], ADD)
        if tt % 2 == 0:
            state["strip"] = psKQT.tile([64, 2, 256], BF16, tag="kqT",
                                        name=f"strip{tt}")
        strip = state["strip"]
        half = (tt % 2) * 128
        nc.tensor.transpose(strip[:, 0, half:half + 128], st[:, 0:64], identB[:])
        nc.tensor.transpose(strip[:, 1, half:half + 128], st[:, 64:128], identB[:])
        if tt % 2 == 1:
            tsl = slice((tt - 1) * 128, (tt + 1) * 128)
            nc.vector.tensor_scalar_add(kqT[:, 0, tsl], strip[:, 0, :], bkq[:, 0:1])
            nc.vector.tensor_scalar_add(kqT[:, 1, tsl], strip[:, 1, :], bkq[:, 1:2])

    e2_of = {}     # (ib, gi) -> e2 tile

    def unit_S(ib, gi):
        group = attn_groups(ib)[gi]
        ncols = group[-1][1] + group[-1][2]
        ps = psS.tile([128, 1024], F32, tag="S", name=f"s{ib}_{gi}")
        for jt, col0, w, _cs in group:
            # diag-odd slot covers i-cols 128:256 of the block only
            ioff = ib * 256 + (128 if w == 128 else 0)
            nc.tensor.matmul(ps[:, col0:col0 + w],
                             kqT[:, 0, jt * 128:(jt + 1) * 128],
                             kqT[:, 1, ioff:ioff + w], start=True, stop=True)
        e2 = e2pool.tile([128, 1024], BF16, tag="e2", name=f"e{ib}_{gi}")
        nc.scalar.activation(e2[:, 0:ncols], ps[:, 0:ncols], EXP)
        if gi == len(attn_groups(ib)) - 1:
            # triangular mask on [diag-even left half | diag-odd right half]
            moff = group[-2][1]
            sel = e2[:, moff:moff + 384].rearrange(
                "p (a c) -> p a c", c=128)[:, 0::2, :]
            nc.vector.tensor_tensor(sel, sel, masks[:], MULT)
        e2_of[(ib, gi)] = e2

    po_of = {}

    def unit_V(ib, gi):
        groups = attn_groups(ib)
        group = groups[gi]
        if gi == 0:
            po_of[ib] = [psO.tile([128, 65], F32, tag="O", name=f"po{ib}_{c}")
                         for c in range(2)]
        po = po_of[ib]
        e2 = e2_of.pop((ib, gi))
        # first/last (group, slot) containing each column c, for start/stop
        first_of, last_of = {}, {}
        for g2, grp in enumerate(groups):
            for s2, (_jt, _c0, _w, cs2) in enumerate(grp):
                for c in cs2:
                    first_of.setdefault(c, (g2, s2))
                    last_of[c] = (g2, s2)
        for si, (jt, col0, w, cs) in enumerate(group):
            for c in cs:
                # e2 chunk covering i-cols c*128:(c+1)*128 of the block
                coff = col0 + (0 if w == 128 else c * 128)
                nc.tensor.matmul(po[c][:], e2[:, coff:coff + 128],
                                 v_aug[:, jt, :],
                                 start=(first_of[c] == (gi, si)),
                                 stop=(last_of[c] == (gi, si)))

    def unit_E(ib):
        po = po_of.pop(ib)
        for c in range(2):
            rec = recpool.tile([128, 1], F32, tag="rec", name=f"rec{ib}_{c}")
            nc.vector.reciprocal(rec[:], po[c][:, 64:65])
            nc.vector.tensor_scalar_mul(o_all[:, 2 * ib + c, :], po[c][:, 0:64],
                                        rec[:])
        nc.sync.dma_start(
            out_d[256 * ib:256 * (ib + 1), :].rearrange("(a p) h -> p a h", p=128),
            o_all[:, 2 * ib:2 * ib + 2, :])

    def emit_schedule():
        s_avail = deque()     # unlocked S units
        v_pend = deque()      # (s_seq_no, ib, gi)
        s_emitted = 0
        ve_done = [0] * NIB   # V groups emitted per ib

        def drain_v(lag):
            nonlocal s_emitted
            while v_pend and v_pend[0][0] <= s_emitted - lag:
                _, ib, gi = v_pend.popleft()
                unit_V(ib, gi)
                ve_done[ib] += 1
                if ve_done[ib] == len(attn_groups(ib)):
                    unit_E(ib)

        for tt in range(NT):
            unit_P(tt)
            if tt % 2 == 1:
                ib = tt // 2
                for gi in range(len(attn_groups(ib))):
                    s_avail.append((ib, gi))
            if s_avail:
                ib, gi = s_avail.popleft()
                unit_S(ib, gi)
                s_emitted += 1
                v_pend.append((s_emitted, ib, gi))
            drain_v(2)
        while s_avail:
            ib, gi = s_avail.popleft()
            unit_S(ib, gi)
            s_emitted += 1
            v_pend.append((s_emitted, ib, gi))
            drain_v(2)
        drain_v(0)

    for rep in range(repeat):
        nc.vector.memset(v_aug[:, :, 64:65], 1.0)
        emit_schedule()


def build_nc(repeat=1):
    nc = bacc.Bacc("TRN2", target_bir_lowering=False, debug=False, num_devices=8)
    xT_d = nc.dram_tensor("xT", [ND, 128, T], BF16, kind="ExternalInput")
    w_d = nc.dram_tensor("w", [ND, 128, 3 * H], BF16, kind="ExternalInput")
    bkq_d = nc.dram_tensor("bkq", [H, 2], F32, kind="ExternalInput")
    bvB_d = nc.dram_tensor("bvB", [128, H], BF16, kind="ExternalInput")
    mask_d = nc.dram_tensor("mask", [128, 2, 128], BF16, kind="ExternalInput")
    out_d = nc.dram_tensor("out", [T, H], F32, kind="ExternalOutput")
    dram = (xT_d, w_d, bkq_d, bvB_d, mask_d, out_d)

    from contextlib import ExitStack
    with tile.TileContext(nc) as tc:
        with ExitStack() as ctx:
            build_body(nc, tc, ctx, dram, repeat=repeat)
    nc.compile()
    return nc


_NC_CACHE = {}


def _get_nc(repeat=1):
    if repeat not in _NC_CACHE:
        _NC_CACHE[repeat] = build_nc(repeat)
    return _NC_CACHE[repeat]


def make_in_maps(x, Wk, bk, Wq, bq, Wv, bv):
    import ml_dtypes
    bf16 = ml_dtypes.bfloat16
    scale = float(H) ** -0.5
    w = np.concatenate([Wk, Wq * scale, Wv], axis=1)          # [512, 192]
    w = np.ascontiguousarray(w.reshape(ND, 128, 3 * H)).astype(bf16)
    bkq = np.ascontiguousarray(np.stack([bk, bq * scale], axis=1)).astype(np.float32)
    bvB = np.ascontiguousarray(np.broadcast_to(bv, (128, H))).astype(bf16)
    r = np.arange(128)[:, None]
    c = np.arange(128)[None, :]
    m = (c >= r).astype(bf16)
    masks = np.ascontiguousarray(np.stack([m, m], axis=1))    # [128, 2, 128]
    ins = []
    for b in range(B):
        xTb = np.ascontiguousarray(x[b].T).astype(bf16).reshape(ND, 128, T)
        ins.append({"xT": xTb, "w": w, "bkq": bkq, "bvB": bvB, "mask": masks})
    return ins


def kernel(x, Wk, bk, Wq, bq, Wv, bv, _repeat=1):
    x = np.asarray(x, dtype=np.float32)
    Wk = np.asarray(Wk, dtype=np.float32)
    bk = np.asarray(bk, dtype=np.float32)
    Wq = np.asarray(Wq, dtype=np.float32)
    bq = np.asarray(bq, dtype=np.float32)
    Wv = np.asarray(Wv, dtype=np.float32)
    bv = np.asarray(bv, dtype=np.float32)

    nc = _get_nc(_repeat)
    in_maps = make_in_maps(x, Wk, bk, Wq, bq, Wv, bv)
    res = run_bass_kernel_spmd(nc, in_maps, core_ids=list(range(B)))
    out = np.stack([res.results[b]["out"] for b in range(B)], axis=0)
    return out.astype(np.float32)


# revision 17
# speedup vs baseline: 1.8861x; 1.1177x over previous
"""Single-head causal self-attention (B=8, T=2048, D=512, H=64), data-parallel
over batch across 8 NeuronCores. Self-contained: builds a Bass/Tile kernel and
runs it via run_bass_kernel_spmd.

v4 design (per core, batch element b = core id), all-bf16 compute:
  - host prep: x transposed + cast to bf16 (xT [4,128,2048]); Wk|Wq*s|Wv
    packed [4,128,192] bf16; biases pre-broadcast (bkqB [128,128],
    bvB [128,64]); triangular mask tile [128,2,128]
  - projections per t-tile: stationary xT chunk [128d,128t] x moving W
    [128d,192] -> psum [128t, k|q|v]; biases folded into the psum->sbuf
    copies (k|q staged for transpose, v+bv -> v_aug); k,q PE-transposed
    (bf16) into [64,2tile] psum strips, then one plain copy to kqT sbuf
  - attention per 256-wide i-block ib (j-tiles 0..2ib+1): S^T groups in
    flat [128,<=1024] psum (no slot crosses a 512-col bank boundary):
    groups of 4 normal tiles; final group = <=2 normals + diag-even full +
    diag-odd right-half (left half fully masked -> skipped); one exp per
    group (ACT) -> e2 bf16; triangular mask multiply on the two diagonal
    128-col pieces (DVE, one strided op)
  - PV: e2 128-col chunks stationary x v_aug [128,65] (v+bv | 1) moving ->
    [t,65] psum; single psum bank [128,2,65]: i-tile c=0 accumulates first
    (group-lagged), then c=1 as a burst - the two accumulation groups are
    temporally disjoint so they share the bank
  - epilogue per i-block: batched reciprocal of ones-columns, scale, DMA out
  - emission is software-pipelined (P/S interleaved, V lagged behind exp);
    the tile scheduler further list-schedules per engine
"""

import sys
from collections import deque

for _p in ("/root/.axon_site/_ro/trn_rl_repo", "/opt/trn_rl_repo"):
    if _p not in sys.path:
        sys.path.append(_p)

import numpy as np
import concourse.bass as bass
import concourse.bacc as bacc
import concourse.tile as tile
from concourse import mybir
from concourse.bass_utils import run_bass_kernel_spmd
from concourse.masks import make_identity

F32 = mybir.dt.float32
BF16 = mybir.dt.bfloat16

B, T, D, H = 8, 2048, 512, 64
NT = T // 128   # 16 t-tiles
ND = D // 128   # 4 d-chunks
NIB = T // 256  # 8 i-blocks
EXP = mybir.ActivationFunctionType.Exp
MULT = mybir.AluOpType.mult
ADD = mybir.AluOpType.add


def attn_groups(ib):
    """Slot lists per group: (jt, col0, width, cs). Normal tiles fill
    groups of 4; final group = <=2 normals + diag-even (256) + diag-odd
    right half (128)."""
    nrm = list(range(2 * ib))
    groups = []
    while len(nrm) > 2:
        groups.append([(nrm.pop(0), i * 256, 256, (0, 1)) for i in range(4)])
    last, col = [], 0
    while nrm:
        last.append((nrm.pop(0), col, 256, (0, 1)))
        col += 256
    last.append((2 * ib, col, 256, (0, 1)))          # diag-even
    last.append((2 * ib + 1, col + 256, 128, (1,)))  # diag-odd right half
    groups.append(last)
    return groups


def build_body(nc, tc, ctx, dram, repeat=1):
    xT_d, w_d, bkqB_d, bvB_d, mask_d, out_d = dram

    persist = ctx.enter_context(tc.tile_pool(name="persist", bufs=1))
    stg = ctx.enter_context(tc.tile_pool(name="stg", bufs=3))
    e2pool = ctx.enter_context(tc.tile_pool(name="e2", bufs=6))
    recpool = ctx.enter_context(tc.tile_pool(name="rec", bufs=2))
    psKQV = ctx.enter_context(tc.tile_pool(name="psKQV", bufs=2, space="PSUM"))
    psKQT = ctx.enter_context(tc.tile_pool(name="psKQT", bufs=1, space="PSUM"))
    psS = ctx.enter_context(tc.tile_pool(name="psS", bufs=2, space="PSUM"))
    psO = ctx.enter_context(tc.tile_pool(name="psO", bufs=1, space="PSUM"))

    identB = persist.tile([128, 128], BF16)
    make_identity(nc, identB[:])

    xT = persist.tile([128, ND, T], BF16)
    w_sb = persist.tile([128, ND, 3 * H], BF16)
    bkqB = persist.tile([128, 128], BF16)
    bvB = persist.tile([128, 64], BF16)
    masks = persist.tile([128, 2, 128], BF16)
    kqT = persist.tile([64, 2, T], BF16)       # [h, {k,q}, t]
    v_aug = persist.tile([128, NT, 65], BF16)  # v rows (+bv) | ones column
    o_all = persist.tile([128, NT, 64], F32)

    # constants first on the scalar HWDGE queue; x tile 0-1 parallel on sync
    nc.scalar.dma_start(w_sb[:], w_d.rearrange("a p h -> p a h"))
    nc.scalar.dma_start(bkqB[:], bkqB_d[:])
    nc.scalar.dma_start(bvB[:], bvB_d[:])
    nc.scalar.dma_start(masks[:], mask_d[:])

    t0 = 0
    for gi, ntile in enumerate((2, 2, 4, 4, 4)):
        eng = nc.sync if gi % 2 == 0 else nc.scalar
        eng.dma_start(
            xT[:, :, 128 * t0:128 * (t0 + ntile)],
            xT_d[:, :, 128 * t0:128 * (t0 + ntile)].rearrange("a p t -> p a t"),
        )
        t0 += ntile

    state = {"strip": None}

    def unit_P(tt):
        ps = psKQV.tile([128, 3 * H], F32, tag="kqv")
        for dc in range(ND):
            nc.tensor.matmul(ps[:], xT[:, dc, tt * 128:(tt + 1) * 128],
                             w_sb[:, dc, :], start=(dc == 0), stop=(dc == ND - 1))
        st = stg.tile([128, 128], BF16, tag="stage", name=f"st{tt}")
        nc.vector.tensor_tensor(st[:], ps[:, 0:128], bkqB[:], ADD)
        nc.vector.tensor_tensor(v_aug[:, tt, 0:64], ps[:, 128:192], bvB[:], ADD)
        if tt % 2 == 0:
            state["strip"] = psKQT.tile([64, 2, 256], BF16, tag="kqT",
                                        name=f"strip{tt}")
        strip = state["strip"]
        half = (tt % 2) * 128
        nc.tensor.transpose(strip[:, 0, half:half + 128], st[:, 0:64], identB[:])
        nc.tensor.transpose(strip[:, 1, half:half + 128], st[:, 64:128], identB[:])
        if tt % 2 == 1:
            nc.vector.tensor_copy(kqT[:, :, (tt - 1) * 128:(tt + 1) * 128],
                                  strip[:])

    e2_of = {}     # (ib, gi) -> e2 tile

    def unit_S(ib, gi):
        groups = attn_groups(ib)
        group = groups[gi]
        ncols = group[-1][1] + group[-1][2]
        ps = psS.tile([128, 1024], F32, tag="S", name=f"s{ib}_{gi}")
        for jt, col0, w, _cs in group:
            ioff = ib * 256 + (128 if w == 128 else 0)
            nc.tensor.matmul(ps[:, col0:col0 + w],
                             kqT[:, 0, jt * 128:(jt + 1) * 128],
                             kqT[:, 1, ioff:ioff + w], start=True, stop=True)
        e2 = e2pool.tile([128, 1024], BF16, tag="e2", name=f"e{ib}_{gi}")
        nc.scalar.activation(e2[:, 0:ncols], ps[:, 0:ncols], EXP)
        if gi == len(groups) - 1:
            moff = group[-2][1]
            sel = e2[:, moff:moff + 384].rearrange(
                "p (a c) -> p a c", c=128)[:, 0::2, :]
            nc.vector.tensor_tensor(sel, sel, masks[:], MULT)
        e2_of[(ib, gi)] = e2

    po_of = {}

    def pv_slots(ib, c):
        """(gi, jt, coff) for every PV matmul of column c, in group order."""
        out = []
        for gi, grp in enumerate(attn_groups(ib)):
            for jt, col0, w, cs in grp:
                if c in cs:
                    out.append((gi, jt, col0 + (0 if w == 128 else c * 128)))
        return out

    def unit_V0(ib, gi):
        """PV pass for i-tile c=0, group gi only (pipelined behind exp)."""
        if gi == 0:
            po_of[ib] = psO.tile([128, 2, 65], F32, tag="O", name=f"po{ib}")
        po = po_of[ib]
        slots = pv_slots(ib, 0)
        mine = [s for s in slots if s[0] == gi]
        for jt, coff in [(j, c) for g, j, c in mine]:
            first = (gi, jt, coff) == slots[0]
            last = (gi, jt, coff) == slots[-1]
            nc.tensor.matmul(po[:, 0, :], e2_of[(ib, gi)][:, coff:coff + 128],
                             v_aug[:, jt, :], start=first, stop=last)

    def unit_V1E(ib):
        """PV pass for c=1 (burst; the c=0 group closed), then epilogue."""
        po = po_of.pop(ib)
        slots = pv_slots(ib, 1)
        for i, (gi, jt, coff) in enumerate(slots):
            nc.tensor.matmul(po[:, 1, :], e2_of[(ib, gi)][:, coff:coff + 128],
                             v_aug[:, jt, :], start=(i == 0),
                             stop=(i == len(slots) - 1))
        for gi in range(len(attn_groups(ib))):
            e2_of.pop((ib, gi))
        rec = recpool.tile([128, 2], F32, tag="rec", name=f"rec{ib}")
        nc.vector.reciprocal(rec[:], po[:, :, 64])
        for c in range(2):
            nc.vector.tensor_scalar_mul(o_all[:, 2 * ib + c, :], po[:, c, 0:64],
                                        rec[:, c:c + 1])
        nc.sync.dma_start(
            out_d[256 * ib:256 * (ib + 1), :].rearrange("(a p) h -> p a h", p=128),
            o_all[:, 2 * ib:2 * ib + 2, :])

    def emit_schedule():
        s_avail = deque()
        v_pend = deque()
        s_emitted = 0
        v0_done = [0] * NIB

        def drain_v(lag):
            while v_pend and v_pend[0][0] <= s_emitted - lag:
                _, ib, gi = v_pend.popleft()
                unit_V0(ib, gi)
                v0_done[ib] += 1
                if v0_done[ib] == len(attn_groups(ib)):
                    unit_V1E(ib)

        for tt in range(NT):
            unit_P(tt)
            if tt % 2 == 1:
                ib = tt // 2
                for gi in range(len(attn_groups(ib))):
                    s_avail.append((ib, gi))
            if s_avail:
                ib, gi = s_avail.popleft()
                unit_S(ib, gi)
                s_emitted += 1
                v_pend.append((s_emitted, ib, gi))
            drain_v(2)
        while s_avail:
            ib, gi = s_avail.popleft()
            unit_S(ib, gi)
            s_emitted += 1
            v_pend.append((s_emitted, ib, gi))
            drain_v(2)
        drain_v(0)

    for rep in range(repeat):
        nc.vector.memset(v_aug[:, :, 64:65], 1.0)
        emit_schedule()


def build_nc(repeat=1):
    nc = bacc.Bacc("TRN2", target_bir_lowering=False, debug=False, num_devices=8)
    xT_d = nc.dram_tensor("xT", [ND, 128, T], BF16, kind="ExternalInput")
    w_d = nc.dram_tensor("w", [ND, 128, 3 * H], BF16, kind="ExternalInput")
    bkqB_d = nc.dram_tensor("bkqB", [128, 128], BF16, kind="ExternalInput")
    bvB_d = nc.dram_tensor("bvB", [128, H], BF16, kind="ExternalInput")
    mask_d = nc.dram_tensor("mask", [128, 2, 128], BF16, kind="ExternalInput")
    out_d = nc.dram_tensor("out", [T, H], F32, kind="ExternalOutput")
    dram = (xT_d, w_d, bkqB_d, bvB_d, mask_d, out_d)

    from contextlib import ExitStack
    with tile.TileContext(nc) as tc:
        with ExitStack() as ctx:
            build_body(nc, tc, ctx, dram, repeat=repeat)
    nc.compile()
    return nc


_NC_CACHE = {}


def _get_nc(repeat=1):
    if repeat not in _NC_CACHE:
        _NC_CACHE[repeat] = build_nc(repeat)
    return _NC_CACHE[repeat]


def make_in_maps(x, Wk, bk, Wq, bq, Wv, bv):
    import ml_dtypes
    bf16 = ml_dtypes.bfloat16
    scale = float(H) ** -0.5
    w = np.concatenate([Wk, Wq * scale, Wv], axis=1)          # [512, 192]
    w = np.ascontiguousarray(w.reshape(ND, 128, 3 * H)).astype(bf16)
    bkq = np.concatenate([bk, bq * scale])                    # [128]
    bkqB = np.ascontiguousarray(np.broadcast_to(bkq, (128, 128))).astype(bf16)
    bvB = np.ascontiguousarray(np.broadcast_to(bv, (128, H))).astype(bf16)
    r = np.arange(128)[:, None]
    c = np.arange(128)[None, :]
    m = (c >= r).astype(bf16)
    masks = np.ascontiguousarray(np.stack([m, m], axis=1))    # [128, 2, 128]
    ins = []
    for b in range(B):
        xTb = np.ascontiguousarray(x[b].T).astype(bf16).reshape(ND, 128, T)
        ins.append({"xT": xTb, "w": w, "bkqB": bkqB, "bvB": bvB, "mask": masks})
    return ins


def kernel(x, Wk, bk, Wq, bq, Wv, bv, _repeat=1):
    x = np.asarray(x, dtype=np.float32)
    Wk = np.asarray(Wk, dtype=np.float32)
    bk = np.asarray(bk, dtype=np.float32)
    Wq = np.asarray(Wq, dtype=np.float32)
    bq = np.asarray(bq, dtype=np.float32)
    Wv = np.asarray(Wv, dtype=np.float32)
    bv = np.asarray(bv, dtype=np.float32)

    nc = _get_nc(_repeat)
    in_maps = make_in_maps(x, Wk, bk, Wq, bq, Wv, bv)
    res = run_bass_kernel_spmd(nc, in_maps, core_ids=list(range(B)))
    out = np.stack([res.results[b]["out"] for b in range(B)], axis=0)
    return out.astype(np.float32)
